# revision 1
# baseline (speedup 1.0000x reference)
"""Multi-head attention (N=4, L=2048, E=1024, H=16) on 8 Trainium2 cores.

Sharding: core c -> (batch n = c // 2, head-group g = c % 2).  Each core
computes, for its batch and its 8 heads (512 embed dims):
  qp_T/kp_T = (W x^T) in [d, tok] layout, vp in [tok, d] layout,
  S_T[k, q] scores with two heads packed in the 128 partitions via PE row
  tiling, exp via ACT with the 1/sqrt(1024) scale folded in, attn@v with a
  ones column appended to vp so the softmax denominator accumulates in the
  same PSUM tile, then the output projection against Wo columns of this
  group.  Host sums the two per-group partial outputs per batch and adds bo.

v2 schedule: the attention inner loop is ACT(exp)-bound (~1.1us per
[128,1024] tile, 256 tiles), so everything else hides behind it:
  - scores(kt) is issued BEFORE attn@v(kt-1) so the in-order PE queue
    never serializes scores behind the exp the attn@v is waiting on.
  - q/k projections for head-pair pr+1 are drip-fed ("fillers") into the
    ACT-bound kt loop of pair pr; normalization + output projection of
    query-block qb are drip-fed into the (pr=3) kt loop of qb+1.
  - softmax denominators are collected into a gather tile and inverted by
    ONE DVE reciprocal per query block -- no Ln/Exp activation-table
    ping-pong on ACT (the old scheme spent 86us in table loads + ln/exp).
  - normalization multiplies read the replicate-matmul PSUM directly.

Matmul operands are fp16 (1 cycle/row on the PE at 2.4 GHz), accumulation
stays fp32 in PSUM.
"""

import os

import numpy as np

import concourse.bacc as bacc
import concourse.mybir as mybir
import concourse.tile as tile
from concourse.bass import ds, ts
from concourse.bass_utils import run_bass_kernel_spmd

F32 = mybir.dt.float32
F16 = mybir.dt.float16

E = 1024          # embed
H = 16            # heads (global)
D = 64            # head dim
L = 2048          # sequence length
NB = 4            # batch
GE = 512          # embed dims per head group (8 heads)
P = 128           # partitions
TB = L // 512     # 4 token blocks of 512
EC = E // P       # 8 embed chunks
DC = GE // P      # 4 d-chunks per group == head pairs
KT = L // P       # 16 key-token chunks

_CACHE = {}


def _build():
    nc = bacc.Bacc("TRN2", debug=False, enable_asserts=False, num_devices=8)

    xq = nc.dram_tensor("xq", [E, L], F16, kind="ExternalInput").ap()
    xk = nc.dram_tensor("xk", [E, L], F16, kind="ExternalInput").ap()
    xv = nc.dram_tensor("xv", [E, L], F16, kind="ExternalInput").ap()
    wq = nc.dram_tensor("wq", [E, GE], F16, kind="ExternalInput").ap()
    wk = nc.dram_tensor("wk", [E, GE], F16, kind="ExternalInput").ap()
    wv = nc.dram_tensor("wv", [E, GE], F16, kind="ExternalInput").ap()
    wo = nc.dram_tensor("wo", [GE, E], F16, kind="ExternalInput").ap()
    bqk = nc.dram_tensor("bqk", [2, P, DC], F32, kind="ExternalInput").ap()
    bvr = nc.dram_tensor("bvr", [1, GE], F16, kind="ExternalInput").ap()
    out = nc.dram_tensor("out", [L, E], F32, kind="ExternalOutput").ap()

    with tile.TileContext(nc) as tc, \
         nc.allow_low_precision(reason="fp16 attention internals by design"):
        with tc.tile_pool(name="persist", bufs=1) as pp, \
             tc.tile_pool(name="wpool", bufs=1) as wp, \
             tc.tile_pool(name="xpool", bufs=3) as xp, \
             tc.tile_pool(name="bias", bufs=1) as bp, \
             tc.tile_pool(name="expp", bufs=4) as ep, \
             tc.tile_pool(name="dinvp", bufs=4) as dp, \
             tc.tile_pool(name="aoup", bufs=4) as au, \
             tc.tile_pool(name="otmp", bufs=2) as ot, \
             tc.tile_pool(name="ppsum", bufs=2, space="PSUM") as pps, \
             tc.tile_pool(name="spsum", bufs=2, space="PSUM") as sps, \
             tc.tile_pool(name="opsum", bufs=1, space="PSUM") as ops:
            # persistent SBUF
            vp = pp.tile([P, KT, 8, D + 1], F16)         # vp_aug per head
            ao = pp.tile([P, DC, L], F16)                # normalized attnout_T
            qs = pp.tile([P, DC, L], F16)                # qp_T  [d, pair, tok]
            ks = pp.tile([P, DC, L], F16)                # kp_T
            ones32 = pp.tile([1, P], F32)
            ones = pp.tile([1, P], F16)
            nc.gpsimd.memset(ones32[:], 1.0)
            nc.vector.tensor_copy(ones[:], ones32[:])

            bq_t = bp.tile([P, DC], F32, tag="bq")
            bk_t = bp.tile([P, DC], F32, tag="bk")
            bv_row = bp.tile([1, GE], F16, tag="bv")
            nc.sync.dma_start(bq_t[:], bqk[0])
            nc.sync.dma_start(bk_t[:], bqk[1])
            nc.sync.dma_start(bv_row[:], bvr)

            wq_sb = wp.tile([P, EC, GE], F16, tag="wq")
            wk_sb = wp.tile([P, EC, GE], F16, tag="wk")
            wv_sb = wp.tile([P, EC, GE], F16, tag="wv")
            wo_sb = wp.tile([P, DC, E], F16, tag="wo")
            nc.sync.dma_start(wq_sb[:], wq.rearrange("(eo p) g -> p eo g", p=P))
            nc.sync.dma_start(wk_sb[:], wk.rearrange("(eo p) g -> p eo g", p=P))
            nc.sync.dma_start(wv_sb[:], wv.rearrange("(eo p) g -> p eo g", p=P))
            nc.sync.dma_start(wo_sb[:], wo.rearrange("(dc p) e -> p dc e", p=P))

            # ---- vp ones column (softmax denominator accumulator) ----
            onescol = bp.tile([P, KT], F32, tag="onescol")
            nc.gpsimd.memset(onescol[:], 1.0)
            nc.vector.tensor_copy(
                vp[:, :, :, D : D + 1],
                onescol[:, :, None, None].to_broadcast([P, KT, 8, 1]),
            )

            # ---------------------------------------------------------------
            # projection machinery: every projection is a "group" (one token
            # slab DMA + its matmuls) expanded into ~1-matmul closures that
            # are either run serially (startup head) or drip-fed with slot
            # deadlines into the ACT(exp)-bound attention loops below.
            # slab DMAs for consecutive groups are chained two ahead.
            # ---------------------------------------------------------------
            slabq = []
            slab_pos = [0]
            pstate = {}

            def issue_next_dma():
                if slab_pos[0] < len(slabq):
                    g = slabq[slab_pos[0]]
                    slab_pos[0] += 1
                    x_sb = xp.tile([P, EC, 512], F16, tag="xslab",
                                   name="x_sb")
                    nc.sync.dma_start(
                        x_sb[:],
                        g["x_ap"][:, ts(g["tb"], 512)].rearrange(
                            "(eo p) t -> p eo t", p=P
                        ),
                    )
                    g["slab"] = x_sb

            def vproj_group(tb):
                """value projection for token block tb: 4 psum sub-blocks of
                128 tokens, each [tok, 512] + bias, copied into vp."""
                g = {"x_ap": xv, "tb": tb}
                slabq.append(g)
                closures = []
                for j in range(4):
                    for e in range(EC):
                        def mm(g=g, j=j, e=e):
                            if j == 0 and e == 0:
                                issue_next_dma()
                            if e == 0:
                                pstate["ps"] = pps.tile(
                                    [P, GE], F32, tag="pp", name="ps"
                                )
                            nc.tensor.matmul(
                                pstate["ps"][:],
                                g["slab"][:, e, ts(j, P)],
                                wv_sb[:, e, :],
                                start=(e == 0),
                                stop=False,
                            )
                        closures.append(mm)

                    def fin(tb=tb, j=j):
                        nc.tensor.matmul(
                            pstate["ps"][:], ones[:, :P], bv_row[:],
                            start=False, stop=True,
                        )
                        nc.vector.tensor_copy(
                            vp[:, tb * 4 + j, :, 0:D],
                            pstate["ps"].rearrange("p (h d) -> p h d", d=D),
                        )
                    closures.append(fin)
                return closures

            def qkproj_group(which, pr, tb):
                """q or k projection for head-pair pr, token block tb."""
                x_ap, w_sb, b_t, st = {
                    "q": (xq, wq_sb, bq_t, qs),
                    "k": (xk, wk_sb, bk_t, ks),
                }[which]
                g = {"x_ap": x_ap, "tb": tb}
                slabq.append(g)
                closures = []
                for e in range(EC):
                    def mm(g=g, e=e, w_sb=w_sb, pr=pr):
                        if e == 0:
                            issue_next_dma()
                            pstate["ps"] = pps.tile([P, 512], F32, tag="pp",
                                                    name="ps")
                        nc.tensor.matmul(
                            pstate["ps"][:],
                            w_sb[:, e, ts(pr, P)],
                            g["slab"][:, e, :],
                            start=(e == 0),
                            stop=(e == EC - 1),
                        )
                    closures.append(mm)

                def badd(b_t=b_t, st=st, pr=pr, tb=tb):
                    nc.vector.tensor_scalar_add(
                        st[:, pr, ts(tb, 512)],
                        pstate["ps"][:],
                        b_t[:, pr : pr + 1],
                    )
                closures.append(badd)
                return closures

            dinvs = {}
            aous = {}
            nstate = {}

            def norm_fillers(pr, qb):
                """normalize the two (pr, qb) blocks: replicate 1/denom
                across 64 partitions via PE, multiply vs the unnormalized
                numerators into ao."""
                fillers = []
                for i in range(2):
                    def repmm(pr=pr, qb=qb, i=i):
                        ps_r = pps.tile([P, 512], F32, tag="pp", name="ps_r")
                        nc.tensor.matmul(
                            ps_r[0:D, :], ones32[:, :D],
                            dinvs.pop((pr, qb, i))[:],
                            start=True, stop=True,
                        )
                        nstate["ps_r"] = ps_r
                    fillers.append(repmm)

                    def mult(pr=pr, qb=qb, i=i):
                        nc.vector.tensor_tensor(
                            ao[ds(D * i, D), pr, ts(qb, 512)],
                            aous.pop((pr, qb, i))[:],
                            nstate["ps_r"][0:D, :],
                            mybir.AluOpType.mult,
                        )
                    fillers.append(mult)
                return fillers

            def outproj_fillers(qb):
                """output projection for query block qb (needs ao of all
                pairs for qb normalized)."""
                fillers = []
                for tc_ in range(4):
                    tok = qb * 4 + tc_
                    for ob in range(2):
                        for dch in range(0, DC, 2):
                            def omm(tok=tok, ob=ob, dch=dch):
                                if ob == 0 and dch == 0:
                                    nstate["ps_f"] = sps.tile(
                                        [P, 1024], F32, tag="sc", name="ps_s"
                                    )
                                for dc in (dch, dch + 1):
                                    nc.tensor.matmul(
                                        nstate["ps_f"][:, ts(ob, 512)],
                                        ao[:, dc, ts(tok, P)],
                                        wo_sb[:, dc, ts(ob, 512)],
                                        start=(dc == 0),
                                        stop=(dc == DC - 1),
                                    )
                            fillers.append(omm)

                    def ocopy(tok=tok):
                        o_t = ot.tile([P, 1024], F32, tag="fout")
                        nc.vector.tensor_copy(o_t[:], nstate["ps_f"][:])
                        nc.sync.dma_start(out[ts(tok, P), :], o_t[:])
                    fillers.append(ocopy)
                return fillers

            # ---- startup head: the minimum serial projection work needed
            # to start attention on pair 0 / query block 0 ----
            head = (vproj_group(0)
                    + [c for tb in range(TB) for c in qkproj_group("k", 0, tb)]
                    + qkproj_group("q", 0, 0))
            # chain-prime two slab DMAs, then run the head serially
            issue_next_dma()
            issue_next_dma()
            for f in head:
                f()

            # filler streams per pair phase: [(deadline_slot, closure)],
            # nondecreasing deadlines, drained in order during attention
            def pr0_stream():
                s = []
                for g in (1, 2, 3):      # vp[tb g] before av of kt=4g
                    cs = vproj_group(g)
                    s += [(4 * (g - 1) + min(3, 4 * i // len(cs)), c)
                          for i, c in enumerate(cs)]
                for g in (1, 2, 3):      # qs[tb g] before query block g
                    cs = qkproj_group("q", 0, g)
                    s += [(16 * (g - 1) + 6 + i, c)
                          for i, c in enumerate(cs)]
                for which in ("q", "k"):
                    for tb in range(TB):
                        s += [(63, c) for c in qkproj_group(which, 1, tb)]
                return s

            def prn_stream(pr):
                s = []
                for which in ("q", "k"):
                    for tb in range(TB):
                        s += [(63, c) for c in qkproj_group(which, pr + 1, tb)]
                return s

            projq, pi = [], 0
            normq, ni = [], 0
            for pr in range(DC):
                if pr == 0:
                    projq, pi = pr0_stream(), 0
                elif pr < DC - 1:
                    projq, pi = prn_stream(pr), 0
                for qb in range(TB):
                    # queue normalization of the previous block pair, and
                    # (last pair) output projection of the previous q block
                    if qb > 0:
                        normq = normq + norm_fillers(pr, qb - 1)
                    elif pr > 0:
                        normq = normq + norm_fillers(pr - 1, TB - 1)
                    if pr == DC - 1 and qb > 0:
                        normq = normq + outproj_fillers(qb - 1)
                    base_n, share_n = ni, len(normq) - ni
                    base_p = pi
                    share_p = -(-(len(projq) - pi) // (TB - qb))
                    ps_oo = [
                        ops.tile([P, 512], F32, tag=f"ov{i}", name=f"ov{i}")
                        for i in range(2)
                    ]
                    e_ts = {}

                    def av(kt, ps_oo=ps_oo, pr=pr):
                        e_t = e_ts.pop(kt)
                        for i in range(2):
                            nc.tensor.matmul(
                                ps_oo[i][0 : D + 1, :],
                                vp[:, kt, 2 * pr + i, :],
                                e_t[:, ts(i, 512)],
                                start=(kt == 0),
                                stop=(kt == KT - 1),
                            )

                    for kt in range(KT):
                        ps_s = sps.tile([P, 1024], F32, tag="sc", name="ps_s")
                        for i in range(2):
                            nc.tensor.matmul(
                                ps_s[:, ts(i, 512)],
                                ks[ds(64 * i, 64), pr, ts(kt, P)],
                                qs[ds(64 * i, 64), pr, ts(qb, 512)],
                                start=True,
                                stop=True,
                                tile_position=(64 * i, 0),
                            )
                        e_t = ep.tile([P, 1024], F16, tag="exp", name="e_t")
                        nc.scalar.activation(
                            e_t[:],
                            ps_s[:],
                            mybir.ActivationFunctionType.Exp,
                            scale=float(1.0 / 32.0),
                        )
                        e_ts[kt] = e_t
                        if kt > 0:
                            av(kt - 1)
                        # drip-feed fillers to fill the ACT-bound gap
                        want = base_n + ((kt + 1) * share_n + KT - 1) // KT
                        while ni < min(want, len(normq)):
                            normq[ni]()
                            ni += 1
                        slot = qb * KT + kt
                        want = base_p + ((kt + 1) * share_p + KT - 1) // KT
                        while pi < len(projq) and (
                            projq[pi][0] <= slot or pi < want
                        ):
                            projq[pi][1]()
                            pi += 1
                    av(KT - 1)

                    # numerators + reciprocal denominators out of PSUM
                    for i in range(2):
                        aot = au.tile([D, 512], F32, tag="aou", name="aot")
                        nc.vector.tensor_copy(aot[:], ps_oo[i][0:D, :])
                        aous[(pr, qb, i)] = aot
                        # the approx reciprocal's exponent-flip seed only
                        # works from a partition-0 SBUF source on hardware:
                        # stage the denominator row there first
                        dn = dp.tile([1, 512], F32, tag="dnst", name="dn")
                        nc.vector.tensor_copy(dn[:], ps_oo[i][D : D + 1, :])
                        dv = dp.tile([1, 512], F32, tag="dinv", name="dv")
                        nc.vector.reciprocal_approx_fast(dv[:], dn[:])
                        dinvs[(pr, qb, i)] = dv
                while pi < len(projq):
                    projq[pi][1]()
                    pi += 1

            # tail: last block pair + last query block's output projection
            for f in (normq[ni:] + norm_fillers(DC - 1, TB - 1)
                      + outproj_fillers(TB - 1)):
                f()

    nc.compile()
    return nc


def kernel(q, k, v, padding_mask, sequence_mask, Wq, bq, Wk, bk, Wv, bv, Wo, bo):
    # masks intentionally unused: the reference discards masked_fill results.
    if "nc" not in _CACHE:
        _CACHE["nc"] = _build()
    nc = _CACHE["nc"]

    q = np.asarray(q, np.float32)
    k = np.asarray(k, np.float32)
    v = np.asarray(v, np.float32)
    Wq = np.asarray(Wq, np.float32)
    Wk = np.asarray(Wk, np.float32)
    Wv = np.asarray(Wv, np.float32)
    Wo = np.asarray(Wo, np.float32)
    bq = np.asarray(bq, np.float32)
    bk = np.asarray(bk, np.float32)
    bv = np.asarray(bv, np.float32)
    bo = np.asarray(bo, np.float32)

    in_maps = []
    for c in range(8):
        n, g = c // 2, c % 2
        sl = slice(g * GE, (g + 1) * GE)
        bqk_arr = np.stack(
            [
                bq[sl].reshape(DC, P).T,
                bk[sl].reshape(DC, P).T,
            ]
        ).astype(np.float32)
        in_maps.append(
            {
                "xq": np.ascontiguousarray(q[n].T.astype(np.float16)),
                "xk": np.ascontiguousarray(k[n].T.astype(np.float16)),
                "xv": np.ascontiguousarray(v[n].T.astype(np.float16)),
                "wq": np.ascontiguousarray(Wq[sl, :].T.astype(np.float16)),
                "wk": np.ascontiguousarray(Wk[sl, :].T.astype(np.float16)),
                "wv": np.ascontiguousarray(Wv[sl, :].T.astype(np.float16)),
                "wo": np.ascontiguousarray(Wo[:, sl].T.astype(np.float16)),
                "bqk": np.ascontiguousarray(bqk_arr),
                "bvr": np.ascontiguousarray(bv[sl][None, :].astype(np.float16)),
            }
        )

    trace = os.environ.get("KERNEL_TRACE") == "1"
    kw = {}
    if trace:
        kw = dict(trace=True, trace_cores=list(range(8)))
    res = run_bass_kernel_spmd(nc, in_maps, core_ids=list(range(8)), **kw)
    if trace:
        _CACHE["exec_time_ns"] = res.exec_time_ns
        _CACHE["mean_exec_time_ns"] = res.mean_exec_time_ns

    outp = np.empty((NB, L, E), np.float32)
    for n in range(NB):
        outp[n] = (
            res.results[2 * n]["out"] + res.results[2 * n + 1]["out"] + bo[None, :]
        )
    return outp



# revision 8
# speedup vs baseline: 1.0474x; 1.0474x over previous
"""Multi-head attention (N=4, L=2048, E=1024, H=16) on 8 Trainium2 cores.

Sharding: core c -> (batch n = c // 2, head-group g = c % 2).  Each core
computes, for its batch and its 8 heads (512 embed dims):
  qp_T/kp_T = (W x^T) in [d, tok] layout, vp in [tok, d] layout,
  S_T[k, q] scores with two heads packed in the 128 partitions via PE row
  tiling, exp via ACT with the 1/sqrt(1024) scale folded in, attn@v, then
  the output projection against Wo columns of this group.  Host sums the
  two per-group partial outputs per batch and adds bo.

v4: the v2 schedule was PE-bound (TensorMatrix ~94% busy, 388us of PE
work vs 281us of ACT exp).  PE work is cut by removing the ones-column
softmax-denominator trick that forced the two per-head attn@v matmuls to
run serially at M=65:
  - attn@v is now two COL-TILED M=64 matmuls (tile_position (0,0) and
    (0,64)) writing one [128, 512] PSUM tile -- the PE runs col tiles
    concurrently, ~halving attn@v wall time.
  - the denominator is accumulated on the idle DVE instead: a running
    fp16 [128, 1024] sum of the e tiles (4 elem/cycle/lane all-SBUF
    16-bit mode), then one K=128/M=1 matmul row-sums it per head, and
    the 1/den replicate matmuls for the two heads col-tile concurrently
    into a single PSUM bank at partitions 0-63 / 64-127, which keeps
    every DVE op partition-aligned.
  - the reciprocal-replicate matmuls stream fp16 (not fp32) rhs.
  - the v-bias add moves off the PE (was a K=1 matmul per token block)
    into the DVE copy of vp via a broadcast bias tile.
  - leaner startup head: only vproj(tb0), kproj(pr0,tb0), qproj(pr0,qb0)
    run serially; the other kproj/vproj groups drip-feed into the qb0
    attention loop with interleaved nondecreasing deadlines.
Matmul operands are fp16, accumulation stays fp32 in PSUM.
"""

import os

import numpy as np

import concourse.bacc as bacc
import concourse.mybir as mybir
import concourse.tile as tile
from concourse.bass import ds, ts
from concourse.bass_utils import run_bass_kernel_spmd

F32 = mybir.dt.float32
F16 = mybir.dt.float16

E = 1024          # embed
H = 16            # heads (global)
D = 64            # head dim
L = 2048          # sequence length
NB = 4            # batch
GE = 512          # embed dims per head group (8 heads)
P = 128           # partitions
TB = L // 512     # 4 token blocks of 512
EC = E // P       # 8 embed chunks
DC = GE // P      # 4 d-chunks per group == head pairs
KT = L // P       # 16 key-token chunks

_CACHE = {}


def _build():
    nc = bacc.Bacc("TRN2", debug=False, enable_asserts=False, num_devices=8)

    xq = nc.dram_tensor("xq", [E, L], F16, kind="ExternalInput").ap()
    xk = nc.dram_tensor("xk", [E, L], F16, kind="ExternalInput").ap()
    xv = nc.dram_tensor("xv", [E, L], F16, kind="ExternalInput").ap()
    wq = nc.dram_tensor("wq", [E, GE], F16, kind="ExternalInput").ap()
    wk = nc.dram_tensor("wk", [E, GE], F16, kind="ExternalInput").ap()
    wv = nc.dram_tensor("wv", [E, GE], F16, kind="ExternalInput").ap()
    wo = nc.dram_tensor("wo", [GE, E], F16, kind="ExternalInput").ap()
    bqk = nc.dram_tensor("bqk", [2, P, DC], F32, kind="ExternalInput").ap()
    bvr = nc.dram_tensor("bvr", [1, GE], F16, kind="ExternalInput").ap()
    out = nc.dram_tensor("out", [L, E], F32, kind="ExternalOutput").ap()

    with tile.TileContext(nc) as tc, \
         nc.allow_low_precision(reason="fp16 attention internals by design"):
        with tc.tile_pool(name="persist", bufs=1) as pp, \
             tc.tile_pool(name="wpool", bufs=1) as wp, \
             tc.tile_pool(name="xpool", bufs=3) as xp, \
             tc.tile_pool(name="bias", bufs=1) as bp, \
             tc.tile_pool(name="expp", bufs=4) as ep, \
             tc.tile_pool(name="denp", bufs=2) as dnp, \
             tc.tile_pool(name="dinvp", bufs=4) as dp, \
             tc.tile_pool(name="aoup", bufs=2) as au, \
             tc.tile_pool(name="otmp", bufs=2) as ot, \
             tc.tile_pool(name="ppsum", bufs=2, space="PSUM") as pps, \
             tc.tile_pool(name="spsum", bufs=2, space="PSUM") as sps, \
             tc.tile_pool(name="opsum", bufs=1, space="PSUM") as ops:
            # persistent SBUF
            vp = pp.tile([P, KT, 8, D], F16)             # vp per head
            ao = pp.tile([P, DC, L], F16)                # normalized attnout_T
            qs = pp.tile([P, DC, L], F16)                # qp_T  [d, pair, tok]
            ks = pp.tile([P, DC, L], F16)                # kp_T
            bv_rep = pp.tile([P, GE], F32)               # bv broadcast to all rows
            ones32 = pp.tile([1, P], F32)
            ones = pp.tile([1, P], F16)
            ones_col = pp.tile([P, 1], F16)              # K=128 ones column
            nc.gpsimd.memset(ones32[:], 1.0)
            nc.vector.tensor_copy(ones[:], ones32[:])
            onescol32 = bp.tile([P, 1], F32, tag="onescol")
            nc.gpsimd.memset(onescol32[:], 1.0)
            nc.vector.tensor_copy(ones_col[:], onescol32[:])

            bq_t = bp.tile([P, DC], F32, tag="bq")
            bk_t = bp.tile([P, DC], F32, tag="bk")
            bv_row = bp.tile([1, GE], F16, tag="bv")
            nc.sync.dma_start(bq_t[:], bqk[0])
            nc.sync.dma_start(bk_t[:], bqk[1])
            nc.sync.dma_start(bv_row[:], bvr)

            wq_sb = wp.tile([P, EC, GE], F16, tag="wq")
            wk_sb = wp.tile([P, EC, GE], F16, tag="wk")
            wv_sb = wp.tile([P, EC, GE], F16, tag="wv")
            wo_sb = wp.tile([P, DC, E], F16, tag="wo")
            nc.sync.dma_start(wv_sb[:], wv.rearrange("(eo p) g -> p eo g", p=P))
            nc.sync.dma_start(wk_sb[:], wk.rearrange("(eo p) g -> p eo g", p=P))
            nc.sync.dma_start(wq_sb[:], wq.rearrange("(eo p) g -> p eo g", p=P))
            nc.sync.dma_start(wo_sb[:], wo.rearrange("(dc p) e -> p dc e", p=P))

            # ---- bv broadcast tile (replaces the per-block bias matmul) ----
            ps_b = pps.tile([P, GE], F32, tag="pp", name="ps_b")
            nc.tensor.matmul(ps_b[:], ones[:, :P], bv_row[:], start=True, stop=True)
            nc.vector.tensor_copy(bv_rep[:], ps_b[:])

            # ---------------------------------------------------------------
            # projection machinery: every projection is a "group" (one token
            # slab DMA + its matmuls) expanded into ~1-matmul closures that
            # are either run serially (startup head) or drip-fed with slot
            # deadlines into the ACT(exp)-bound attention loops below.
            # slab DMAs for consecutive groups are chained two ahead.
            # ---------------------------------------------------------------
            slabq = []
            slab_pos = [0]
            pstate = {}

            def issue_next_dma():
                if slab_pos[0] < len(slabq):
                    g = slabq[slab_pos[0]]
                    slab_pos[0] += 1
                    x_sb = xp.tile([P, EC, 512], F16, tag="xslab",
                                   name="x_sb")
                    nc.sync.dma_start(
                        x_sb[:],
                        g["x_ap"][:, ts(g["tb"], 512)].rearrange(
                            "(eo p) t -> p eo t", p=P
                        ),
                    )
                    g["slab"] = x_sb

            def vproj_group(tb):
                """value projection for token block tb: 4 psum sub-blocks of
                128 tokens, each [tok, 512]; bias added in the DVE copy."""
                g = {"x_ap": xv, "tb": tb}
                slabq.append(g)
                closures = []
                for j in range(4):
                    for e in range(EC):
                        def mm(g=g, j=j, e=e):
                            if j == 0 and e == 0:
                                issue_next_dma()
                            if e == 0:
                                pstate["ps"] = pps.tile(
                                    [P, GE], F32, tag="pp", name="ps"
                                )
                            nc.tensor.matmul(
                                pstate["ps"][:],
                                g["slab"][:, e, ts(j, P)],
                                wv_sb[:, e, :],
                                start=(e == 0),
                                stop=(e == EC - 1),
                            )
                        closures.append(mm)

                    def fin(tb=tb, j=j):
                        nc.vector.tensor_tensor(
                            vp[:, tb * 4 + j, :, :],
                            pstate["ps"].rearrange("p (h d) -> p h d", d=D),
                            bv_rep.rearrange("p (h d) -> p h d", d=D),
                            mybir.AluOpType.add,
                        )
                    closures.append(fin)
                return closures

            def qkproj_group(which, pr, tb):
                """q or k projection for head-pair pr, token block tb."""
                x_ap, w_sb, b_t, st = {
                    "q": (xq, wq_sb, bq_t, qs),
                    "k": (xk, wk_sb, bk_t, ks),
                }[which]
                g = {"x_ap": x_ap, "tb": tb}
                slabq.append(g)
                closures = []
                for e in range(EC):
                    def mm(g=g, e=e, w_sb=w_sb, pr=pr):
                        if e == 0:
                            issue_next_dma()
                            pstate["ps"] = pps.tile([P, 512], F32, tag="pp",
                                                    name="ps")
                        nc.tensor.matmul(
                            pstate["ps"][:],
                            w_sb[:, e, ts(pr, P)],
                            g["slab"][:, e, :],
                            start=(e == 0),
                            stop=(e == EC - 1),
                        )
                    closures.append(mm)

                def badd(b_t=b_t, st=st, pr=pr, tb=tb):
                    nc.vector.tensor_scalar_add(
                        st[:, pr, ts(tb, 512)],
                        pstate["ps"][:],
                        b_t[:, pr : pr + 1],
                    )
                closures.append(badd)
                return closures

            dinvs = {}
            aous = {}
            nstate = {}

            def norm_fillers(pr, qb):
                """normalize the two (pr, qb) blocks: replicate 1/denom for
                head i across partitions 64i..64i+63 via two col-tiled PE
                matmuls into ONE psum bank, multiply vs the unnormalized
                numerators into ao (all partition-aligned)."""
                fillers = []

                def repmm(pr=pr, qb=qb):
                    ps_r = pps.tile([P, 512], F32, tag="pp", name="ps_r")
                    for i in range(2):
                        nc.tensor.matmul(
                            ps_r[ds(D * i, D), :], ones[:, :D],
                            dinvs.pop((pr, qb, i))[:],
                            start=True, stop=True,
                            tile_position=(0, D * i),
                        )
                    nstate["ps_r"] = ps_r
                fillers.append(repmm)

                for i in range(2):
                    def mult(pr=pr, qb=qb, i=i):
                        nc.vector.tensor_tensor(
                            ao[ds(D * i, D), pr, ts(qb, 512)],
                            aous[(pr, qb)][ds(D * i, D), :],
                            nstate["ps_r"][ds(D * i, D), :],
                            mybir.AluOpType.mult,
                        )
                    fillers.append(mult)

                def drop(pr=pr, qb=qb):
                    aous.pop((pr, qb))
                fillers.append(drop)
                return fillers

            def outproj_fillers(qb):
                """output projection for query block qb (needs ao of all
                pairs for qb normalized)."""
                fillers = []
                for tc_ in range(4):
                    tok = qb * 4 + tc_
                    for ob in range(2):
                        for dch in range(0, DC, 2):
                            def omm(tok=tok, ob=ob, dch=dch):
                                if ob == 0 and dch == 0:
                                    nstate["ps_f"] = sps.tile(
                                        [P, 1024], F32, tag="sc", name="ps_s"
                                    )
                                for dc in (dch, dch + 1):
                                    nc.tensor.matmul(
                                        nstate["ps_f"][:, ts(ob, 512)],
                                        ao[:, dc, ts(tok, P)],
                                        wo_sb[:, dc, ts(ob, 512)],
                                        start=(dc == 0),
                                        stop=(dc == DC - 1),
                                    )
                            fillers.append(omm)

                    def ocopy(tok=tok):
                        o_t = ot.tile([P, 1024], F32, tag="fout")
                        nc.vector.tensor_copy(o_t[:], nstate["ps_f"][:])
                        nc.sync.dma_start(out[ts(tok, P), :], o_t[:])
                    fillers.append(ocopy)
                return fillers

            # ---- startup head: the minimum serial projection work needed
            # to start attention on pair 0 / query block 0 ----
            head = (vproj_group(0)
                    + qkproj_group("k", 0, 0)
                    + qkproj_group("q", 0, 0))
            # chain-prime two slab DMAs, then run the head serially
            issue_next_dma()
            issue_next_dma()
            for f in head:
                f()

            # filler streams per pair phase: [(deadline_slot, closure)],
            # nondecreasing deadlines, drained in order during attention
            def pr0_stream():
                s = []
                for g in (1, 2, 3):
                    # ks[tb g] before scores of kt=4g, vp[tb g] before av
                    # of kt=4g: interleaved so deadlines stay nondecreasing
                    cs = qkproj_group("k", 0, g) + vproj_group(g)
                    s += [(4 * (g - 1) + min(3, 4 * i // len(cs)), c)
                          for i, c in enumerate(cs)]
                for g in (1, 2, 3):      # qs[tb g] before query block g
                    cs = qkproj_group("q", 0, g)
                    s += [(16 * (g - 1) + 6 + i, c)
                          for i, c in enumerate(cs)]
                for which in ("q", "k"):
                    for tb in range(TB):
                        s += [(63, c) for c in qkproj_group(which, 1, tb)]
                return s

            def prn_stream(pr):
                s = []
                for which in ("q", "k"):
                    for tb in range(TB):
                        s += [(63, c) for c in qkproj_group(which, pr + 1, tb)]
                return s

            projq, pi = [], 0
            normq, ni = [], 0
            for pr in range(DC):
                if pr == 0:
                    projq, pi = pr0_stream(), 0
                elif pr < DC - 1:
                    projq, pi = prn_stream(pr), 0
                for qb in range(TB):
                    # queue normalization of the previous block pair, and
                    # (last pair) output projection of the previous q block
                    if qb > 0:
                        normq = normq + norm_fillers(pr, qb - 1)
                    elif pr > 0:
                        normq = normq + norm_fillers(pr - 1, TB - 1)
                    if pr == DC - 1 and qb > 0:
                        normq = normq + outproj_fillers(qb - 1)
                    base_n, share_n = ni, len(normq) - ni
                    base_p = pi
                    share_p = -(-(len(projq) - pi) // (TB - qb))
                    # one PSUM bank per head; head i writes partitions
                    # 64i..64i+63 of its own bank so the two matmuls land in
                    # different col groups and stream concurrently, while
                    # each bank keeps a private accumulation-group state
                    ps_oo = [
                        ops.tile([P, 512], F32, tag=f"ov{i}", name=f"ov{i}")
                        for i in range(2)
                    ]
                    acc = dnp.tile([P, 1024], F16, tag="den", name="acc")
                    e_ts = {}

                    def av(kt, ps_oo=ps_oo, pr=pr):
                        e_t = e_ts.pop(kt)
                        for i in range(2):
                            nc.tensor.matmul(
                                ps_oo[i][ds(D * i, D), :],
                                vp[:, kt, 2 * pr + i, :],
                                e_t[:, ts(i, 512)],
                                start=(kt == 0),
                                stop=(kt == KT - 1),
                                tile_position=(0, D * i),
                            )

                    for kt in range(KT):
                        ps_s = sps.tile([P, 1024], F32, tag="sc", name="ps_s")
                        for i in range(2):
                            nc.tensor.matmul(
                                ps_s[:, ts(i, 512)],
                                ks[ds(64 * i, 64), pr, ts(kt, P)],
                                qs[ds(64 * i, 64), pr, ts(qb, 512)],
                                start=True,
                                stop=True,
                                tile_position=(64 * i, 0),
                            )
                        e_t = ep.tile([P, 1024], F16, tag="exp", name="e_t")
                        nc.scalar.activation(
                            e_t[:],
                            ps_s[:],
                            mybir.ActivationFunctionType.Exp,
                            scale=float(1.0 / 32.0),
                        )
                        e_ts[kt] = e_t
                        # denominator partial sums on the DVE (fp16 4x mode)
                        if kt == 0:
                            nc.vector.tensor_copy(acc[:], e_t[:])
                        else:
                            nc.vector.tensor_tensor(
                                acc[:], acc[:], e_t[:], mybir.AluOpType.add
                            )
                        if kt > 0:
                            av(kt - 1)
                        # drip-feed fillers to fill the ACT-bound gap
                        want = base_n + ((kt + 1) * share_n + KT - 1) // KT
                        while ni < min(want, len(normq)):
                            normq[ni]()
                            ni += 1
                        slot = qb * KT + kt
                        want = base_p + ((kt + 1) * share_p + KT - 1) // KT
                        while pi < len(projq) and (
                            projq[pi][0] <= slot or pi < want
                        ):
                            projq[pi][1]()
                            pi += 1
                    av(KT - 1)

                    # numerators out of PSUM (partition-aligned per head)
                    aou = au.tile([P, 512], F32, tag="aou", name="aou")
                    for i in range(2):
                        nc.vector.tensor_copy(
                            aou[ds(D * i, D), :], ps_oo[i][ds(D * i, D), :]
                        )
                    aous[(pr, qb)] = aou
                    # denominators: row-sum acc via K=128/M=1 matmuls, then
                    # reciprocal from a partition-0 SBUF staging row
                    for i in range(2):
                        dps = pps.tile([1, 512], F32, tag="pp", name="dps")
                        nc.tensor.matmul(
                            dps[:], ones_col[:], acc[:, ts(i, 512)],
                            start=True, stop=True,
                        )
                        dn = dp.tile([1, 512], F32, tag="dnst", name="dn")
                        nc.vector.tensor_copy(dn[:], dps[:])
                        dv32 = dp.tile([1, 512], F32, tag="dinv", name="dv32")
                        nc.vector.reciprocal_approx_fast(dv32[:], dn[:])
                        dv = dp.tile([1, 512], F16, tag="dinv16", name="dv")
                        nc.vector.tensor_copy(dv[:], dv32[:])
                        dinvs[(pr, qb, i)] = dv
                while pi < len(projq):
                    projq[pi][1]()
                    pi += 1

            # tail: last block pair + last query block's output projection
            for f in (normq[ni:] + norm_fillers(DC - 1, TB - 1)
                      + outproj_fillers(TB - 1)):
                f()

    nc.compile()
    return nc


def kernel(q, k, v, padding_mask, sequence_mask, Wq, bq, Wk, bk, Wv, bv, Wo, bo):
    # masks intentionally unused: the reference discards masked_fill results.
    if "nc" not in _CACHE:
        _CACHE["nc"] = _build()
    nc = _CACHE["nc"]

    q = np.asarray(q, np.float32)
    k = np.asarray(k, np.float32)
    v = np.asarray(v, np.float32)
    Wq = np.asarray(Wq, np.float32)
    Wk = np.asarray(Wk, np.float32)
    Wv = np.asarray(Wv, np.float32)
    Wo = np.asarray(Wo, np.float32)
    bq = np.asarray(bq, np.float32)
    bk = np.asarray(bk, np.float32)
    bv = np.asarray(bv, np.float32)
    bo = np.asarray(bo, np.float32)

    in_maps = []
    for c in range(8):
        n, g = c // 2, c % 2
        sl = slice(g * GE, (g + 1) * GE)
        bqk_arr = np.stack(
            [
                bq[sl].reshape(DC, P).T,
                bk[sl].reshape(DC, P).T,
            ]
        ).astype(np.float32)
        in_maps.append(
            {
                "xq": np.ascontiguousarray(q[n].T.astype(np.float16)),
                "xk": np.ascontiguousarray(k[n].T.astype(np.float16)),
                "xv": np.ascontiguousarray(v[n].T.astype(np.float16)),
                "wq": np.ascontiguousarray(Wq[sl, :].T.astype(np.float16)),
                "wk": np.ascontiguousarray(Wk[sl, :].T.astype(np.float16)),
                "wv": np.ascontiguousarray(Wv[sl, :].T.astype(np.float16)),
                "wo": np.ascontiguousarray(Wo[:, sl].T.astype(np.float16)),
                "bqk": np.ascontiguousarray(bqk_arr),
                "bvr": np.ascontiguousarray(bv[sl][None, :].astype(np.float16)),
            }
        )

    trace = os.environ.get("KERNEL_TRACE") == "1"
    kw = {}
    if trace:
        kw = dict(trace=True, trace_cores=list(range(8)))
    res = run_bass_kernel_spmd(nc, in_maps, core_ids=list(range(8)), **kw)
    if trace:
        _CACHE["exec_time_ns"] = res.exec_time_ns
        _CACHE["mean_exec_time_ns"] = res.mean_exec_time_ns

    outp = np.empty((NB, L, E), np.float32)
    for n in range(NB):
        outp[n] = (
            res.results[2 * n]["out"] + res.results[2 * n + 1]["out"] + bo[None, :]
        )
    return outp
